# revision 3
# baseline (speedup 1.0000x reference)
"""Conv1d (B=32, C_in=C_out=256, W=4096, K=3, pad=1) on 8 Trainium2 cores.

Strategy: data-parallel over batch (4 per core). Per core the conv is a sum
of 6 accumulated matmuls per output tile: contraction over (tap u in 0..2,
ci_chunk in 0..1) with lhsT = weight[ci_chunk, :, co_chunk, u].T ([128 ci x
128 co]) and rhs = padded-x slice [128 ci x 512 positions]. bf16 inputs,
fp32 PSUM accumulation, bias added during the PSUM->SBUF drain.
"""

import numpy as np
import ml_dtypes

BF16 = ml_dtypes.bfloat16

B, C, W, K = 32, 256, 4096, 3
NCORES = 8
BPC = B // NCORES          # batches per core
P = 128                    # partitions
CIC = C // P               # ci chunks
COC = C // P               # co chunks
NCH = 512                  # positions per matmul (one PSUM bank of fp32)
NCHUNKS = W // NCH         # position chunks

_cache = {}


def _build_program():
    import concourse.bass as bass
    import concourse.bacc as bacc
    import concourse.mybir as mybir
    from concourse import tile

    nc = bacc.Bacc(None, target_bir_lowering=False)
    xp_d = nc.dram_tensor("xp", [BPC, CIC, P, W + 2], mybir.dt.bfloat16,
                          kind="ExternalInput")
    w_d = nc.dram_tensor("wt", [P, K * CIC * COC, P], mybir.dt.bfloat16,
                         kind="ExternalInput")
    b_d = nc.dram_tensor("bb", [P, COC], mybir.dt.float32,
                         kind="ExternalInput")
    out_d = nc.dram_tensor("out", [BPC, COC, P, W], mybir.dt.float32,
                           kind="ExternalOutput")

    with tile.TileContext(nc) as tc:
        with (
            tc.tile_pool(name="wp", bufs=1) as wp,
            tc.tile_pool(name="xpool", bufs=BPC * CIC) as xpool,
            tc.tile_pool(name="opool", bufs=3) as opool,
            tc.tile_pool(name="pspool", bufs=8, space=bass.MemorySpace.PSUM) as pspool,
        ):
            w_sb = wp.tile([P, K * CIC * COC, P], mybir.dt.bfloat16)
            nc.sync.dma_start(w_sb[:], w_d[:])
            b_sb = wp.tile([P, COC], mybir.dt.float32)
            nc.sync.dma_start(b_sb[:], b_d[:])

            x_sb = {}
            for b in range(BPC):
                for ci in range(CIC):
                    t = xpool.tile([P, W + 2], mybir.dt.bfloat16)
                    nc.sync.dma_start(t[:], xp_d[b, ci])
                    x_sb[(b, ci)] = t

            for b in range(BPC):
                for co in range(COC):
                    o_sb = opool.tile([P, W], mybir.dt.float32)
                    for n in range(NCHUNKS):
                        ps = pspool.tile([P, NCH], mybir.dt.float32)
                        k = 0
                        nacc = K * CIC
                        for u in range(K):
                            for ci in range(CIC):
                                nc.tensor.matmul(
                                    ps[:],
                                    w_sb[:, (u * CIC + ci) * COC + co, :],
                                    x_sb[(b, ci)][:, n * NCH + u:n * NCH + u + NCH],
                                    start=(k == 0),
                                    stop=(k == nacc - 1),
                                )
                                k += 1
                        nc.any.tensor_scalar_add(
                            o_sb[:, n * NCH:(n + 1) * NCH], ps[:],
                            b_sb[:, co:co + 1],
                        )
                    nc.sync.dma_start(out_d[b, co], o_sb[:])
    nc.compile()
    return nc


def _prep_inputs(x, weight, bias):
    # x: [32,256,4096] f32 -> per-core padded bf16 [BPC, CIC, 128, W+2]
    xp = np.zeros((B, CIC, P, W + 2), BF16)
    xp[:, :, :, 1:W + 1] = x.reshape(B, CIC, P, W).astype(BF16)
    # weight: [co, ci, u] -> [ci_in, (u, ci_c, co_c), co_in]
    wt = weight.reshape(COC, P, CIC, P, K)          # [co_c, co_in, ci_c, ci_in, u]
    w_host = np.ascontiguousarray(
        wt.transpose(3, 4, 2, 0, 1)                 # [ci_in, u, ci_c, co_c, co_in]
    ).reshape(P, K * CIC * COC, P).astype(BF16)
    b_host = np.ascontiguousarray(bias.reshape(COC, P).T).astype(np.float32)
    return xp, w_host, b_host


def run(x, weight, bias, trace=False):
    from concourse.bass_utils import run_bass_kernel_spmd

    if "nc" not in _cache:
        _cache["nc"] = _build_program()
    nc = _cache["nc"]

    xp, w_host, b_host = _prep_inputs(
        np.asarray(x, np.float32), np.asarray(weight, np.float32),
        np.asarray(bias, np.float32))
    in_maps = [
        {"xp": xp[c * BPC:(c + 1) * BPC], "wt": w_host, "bb": b_host}
        for c in range(NCORES)
    ]
    res = run_bass_kernel_spmd(nc, in_maps, list(range(NCORES)), trace=trace)
    out = np.concatenate(
        [res.results[c]["out"].reshape(BPC, C, W) for c in range(NCORES)], axis=0)
    return out, res


def kernel(x, weight, bias):
    out, _ = run(x, weight, bias, trace=False)
    return out
